# revision 10
# baseline (speedup 1.0000x reference)
"""Deformable Conv (DCNv1) Trainium2 Bass kernel — v2.

Problem: x[4,64,128,128], offset[4,18,128,128], weight[64,64,3,3], bias[64]
-> out[4,64,128,128].  3x3 deformable conv, stride 1, pad 1, bilinear sampling.

Sharding: 8 cores = (batch b = core//2) x (H-half h = core%2). Each core
computes out[b, :, 64h:64h+64, :].

Host prep (layout only): per-core bf16 "patch table" T[(y',x'), (c,r)] where
row (y',x') holds img rows y'-18+64h and y'-17+64h at col x'-16, channel-major
with the two rows interleaved innermost.  One 512B gather chunk starting at
row (y',x') covers the full 2x2 bilinear patch x 64 channels.  Offsets are
shipped pre-transposed as off_T[wo, (ho, ch)], weights as bf16 chunk-packed
[128, 5*64] in (k,c)-major contraction order.

Device per-core:
  1. DVE: sampling positions p = off + iota (one floor computation feeds BOTH
     the bilinear weights and the gather indices - no dual-path mismatch).
     idx32[wo, (ho,k)] = ((y0+18)*160 + x0+16)*128; w4[wo, (ho,k,x,r)] bf16.
  2. Per 4-row group: one gpsimd.indirect_dma_start gathers
     v4[wo, (row,k), 256] (4608 descriptors x 512B, HW dynamic queue).
  3. DVE blend (2x-mode layouts): m = v4*w4 (bcast over c, r innermost),
     x-fold add, r-fold add -> s2[wo, (row, k*64+c)] bf16.
  4. PE: 5 transposes/row -> st[(kc), (row,wo)]; 5 GEMMs of free dim 512
     accumulating the 576-deep contraction in PSUM; Act adds bias; DMA out.
"""

import numpy as np
import sys

sys.path.insert(0, "/opt/trn_rl_repo")

import ml_dtypes
import concourse.bacc as bacc
import concourse.mybir as mybir
from concourse import tile
from concourse.bass import AP
from concourse.bass_utils import run_bass_kernel_spmd
from concourse.masks import make_identity
from concourse.library_config import mlp

# problem constants
B, C, H, W = 4, 64, 128, 128
K, O = 9, 64
HO2 = 64            # output rows per core
NY = 92             # table y rows (image rows 64h-18 .. 64h+73)
WP = 160            # table x cols (image cols -16 .. 143)
TROWS = NY * WP
KC = K * C          # 576 im2col contraction
NCHUNK = 5          # ceil(576/128)
RG = 4              # output rows per group
NG = HO2 // RG
F32 = mybir.dt.float32
BF16 = mybir.dt.bfloat16
I32 = mybir.dt.int32
I16 = mybir.dt.int16
AX = mybir.AluOpType

_CACHE = {}


def _build_nc():
    nc = bacc.Bacc("TRN2", target_bir_lowering=False, debug=False,
                   num_swdge_queues=4)

    tbl = nc.dram_tensor("tbl", [TROWS + 2, 2 * C], BF16, kind="ExternalInput").ap()
    offT_d = nc.dram_tensor("offT", [128, HO2 * 18], F32, kind="ExternalInput").ap()
    wt_d = nc.dram_tensor("wt", [128, NCHUNK * O], BF16, kind="ExternalInput").ap()
    bias_d = nc.dram_tensor("bias", [O, 1], F32, kind="ExternalInput").ap()
    out_d = nc.dram_tensor("out", [O, HO2, W], F32, kind="ExternalOutput").ap()

    with tile.TileContext(nc) as tc:
        with (
            tc.tile_pool(name="consts", bufs=1) as consts,
            tc.tile_pool(name="wmt", bufs=10) as wmt,
            tc.tile_pool(name="v4p", bufs=4) as v4p,
            tc.tile_pool(name="mp", bufs=1) as mp,
            tc.tile_pool(name="tp", bufs=1) as tp,
            tc.tile_pool(name="sp", bufs=2) as sp,
            tc.tile_pool(name="stp", bufs=2) as stp,
            tc.tile_pool(name="outp", bufs=2) as outp,
            tc.tile_pool(name="ps_tr", bufs=4, space="PSUM") as ps_tr,
            tc.tile_pool(name="ps_mm", bufs=2, space="PSUM") as ps_mm,
        ):
            ident = consts.tile([128, 128], BF16)
            identf = consts.tile([128, 128], F32)

            offT = consts.tile([128, HO2 * 18], F32)
            nc.sync.dma_start(offT, offT_d)
            wt_bf = consts.tile([128, NCHUNK * O], BF16)
            nc.sync.dma_start(wt_bf, wt_d)
            bias_sb = consts.tile([O, 1], F32)
            nc.sync.dma_start(bias_sb, bias_d)

            # iota: cy = ki + ho + 15, cx = kj + wo + 15  (both [wo, (ho,k)])
            cy_i = wmt.tile([128, 576], I32, tag="wm")
            nc.gpsimd.iota(cy_i, pattern=[[1, 64], [1, 3], [0, 3]], base=15,
                           channel_multiplier=0)
            cy = consts.tile([128, 576], F32)
            nc.vector.tensor_copy(cy, cy_i)
            cx_i = wmt.tile([128, 576], I32, tag="wm")
            nc.gpsimd.iota(cx_i, pattern=[[0, 64], [0, 3], [1, 3]], base=15,
                           channel_multiplier=1)
            cx = consts.tile([128, 576], F32)
            nc.vector.tensor_copy(cx, cx_i)
            nc.gpsimd.load_library(mlp)
            make_identity(nc, ident)
            make_identity(nc, identf)

            # sampling positions (+16 folded into iota bases; exact floor via
            # round-to-int + is_gt correction)
            offv = offT.rearrange("p (h k two) -> p h k two", k=9, two=2)
            cyv = cy.rearrange("p (h k) -> p h k", k=9)
            cxv = cx.rearrange("p (h k) -> p h k", k=9)

            def setup(name, shape=None, dt=F32):
                return wmt.tile(shape or [128, 576], dt, tag="wm", name=name)

            pys = setup("pys")
            nc.vector.tensor_tensor(pys.rearrange("p (h k) -> p h k", k=9),
                                    offv[:, :, :, 0], cyv, AX.add)
            pxs = setup("pxs")
            nc.vector.tensor_tensor(pxs.rearrange("p (h k) -> p h k", k=9),
                                    offv[:, :, :, 1], cxv, AX.add)
            y0i = setup("y0i", dt=I32)
            nc.vector.tensor_copy(y0i, pys)
            y0f = setup("y0f")
            nc.vector.tensor_copy(y0f, y0i)
            cr = setup("cr")
            nc.vector.tensor_tensor(cr, y0f, pys, AX.is_gt)
            nc.vector.tensor_tensor(y0f, y0f, cr, AX.subtract)
            x0i = setup("x0i", dt=I32)
            nc.vector.tensor_copy(x0i, pxs)
            x0f = setup("x0f")
            nc.vector.tensor_copy(x0f, x0i)
            nc.vector.tensor_tensor(cr, x0f, pxs, AX.is_gt)
            nc.vector.tensor_tensor(x0f, x0f, cr, AX.subtract)
            ly = setup("ly")
            nc.vector.tensor_tensor(ly, pys, y0f, AX.subtract)
            lx = setup("lx")
            nc.vector.tensor_tensor(lx, pxs, x0f, AX.subtract)
            hy = setup("hy")
            nc.vector.tensor_scalar(hy, ly, -1.0, 1.0, AX.mult, AX.add)
            hx = setup("hx")
            nc.vector.tensor_scalar(hx, lx, -1.0, 1.0, AX.mult, AX.add)

            # corner weights w4[wo, (ho, k, x, r)] bf16
            w4 = consts.tile([128, 2304], BF16)
            w4v = w4.rearrange("p (a x r) -> p a x r", x=2, r=2)
            nc.vector.tensor_tensor(w4v[:, :, 0, 0], hy, hx, AX.mult)
            nc.vector.tensor_tensor(w4v[:, :, 0, 1], ly, hx, AX.mult)
            nc.vector.tensor_tensor(w4v[:, :, 1, 0], hy, lx, AX.mult)
            nc.vector.tensor_tensor(w4v[:, :, 1, 1], ly, lx, AX.mult)

            # gather indices: (y0f+2 clip [0,90])*160 + (x0f clip [0,158])
            yq = setup("yq")
            nc.vector.tensor_scalar(yq, y0f, 2.0, 0.0, AX.add, AX.max)
            nc.vector.tensor_scalar_min(yq, yq, 90.0)
            xq = setup("xq")
            nc.vector.tensor_scalar(xq, x0f, 0.0, 158.0, AX.max, AX.min)
            idxf = setup("idxf")
            nc.vector.scalar_tensor_tensor(idxf, yq, float(WP), xq,
                                           op0=AX.mult, op1=AX.add)
            # wrap for dma_gather: idx i (i = a*128 + wo, a = ho*9+k) lives at
            # wrapped[wo%16, a*8 + wo//16], replicated across 16-partition
            # groups.  Shuffle the FLOORED f32 values (single floor path),
            # then permute+convert to int16 on DVE, then replicate via DMA.
            wfd = consts.tile([16, 8 * 576], F32, name="wfd")
            for d in range(8):
                nc.sync.dma_start(wfd[:, d * 576 : (d + 1) * 576],
                                  idxf[16 * d : 16 * d + 16, :])
            idxw = consts.tile([128, 4608], I16, name="idxw")
            nc.vector.tensor_copy(
                idxw[:16, :].rearrange("p (a d) -> p a d", d=8),
                wfd.rearrange("p (d a) -> p a d", a=576))
            for rep in range(1, 8):
                nc.sync.dma_start(idxw[16 * rep : 16 * rep + 16, :],
                                  idxw[:16, :])

            # ---------------- main loop ----------------
            w4g_all = w4.rearrange("p (h k x r) -> p h k x r", k=9, x=2, r=2)
            tbl_ov = AP(tbl.tensor, 0, [[128, TROWS], [1, 256]])
            GW = RG * 9 * 8    # wrapped idx cols per group (4608/16)
            for g in range(NG):
                v4 = v4p.tile([128, RG * 9, 256], BF16, tag="v4")
                nc.gpsimd.dma_gather(
                    v4, tbl_ov, idxw[:, g * GW : (g + 1) * GW],
                    RG * 9 * 128, RG * 9 * 128, 256, elem_step=128,
                    single_packet=False, queue_num=g % 4)
                # blend: m = v4 * w4 (bcast c), chunk content = (x, c, r)
                v4v = v4.rearrange("p a (x c r) -> p a x c r", x=2, c=64, r=2)
                w4b = (w4g_all[:, g * RG : (g + 1) * RG]
                       .rearrange("p h k x r -> p (h k) x r")
                       [:, :, :, None, :].to_broadcast((128, RG * 9, 2, 64, 2)))
                m = mp.tile([128, RG * 2304], BF16, tag="m")
                mv = m.rearrange("p (a x c r) -> p a x c r", x=2, c=64, r=2)
                nc.vector.tensor_tensor(mv, v4v, w4b, AX.mult)
                # x-fold then r-fold
                t = tp.tile([128, RG * 1152], BF16, tag="t")
                tv = t.rearrange("p (a c r) -> p a c r", c=64, r=2)
                nc.vector.tensor_tensor(tv, mv[:, :, 0], mv[:, :, 1], AX.add)
                # r-fold into f32-packed row pairs: s2 f32[p,(pair,kc)],
                # bf16 view slot = row parity.  PE transposes the packed f32
                # (bit-exact, probe-verified), halving transpose count.
                s2 = sp.tile([128, (RG // 2) * 576], F32, tag="s")
                s2b = s2.bitcast(BF16)
                with nc.allow_low_precision(reason="bilinear r-fold"):
                    for q in range(RG // 2):
                        nc.vector.reduce_sum(
                            s2b[:, q * 1152 : (q + 1) * 1152].rearrange(
                                "p (kc sl) -> p sl kc", sl=2),
                            t[:, 2 * q * 1152 : (2 * q + 2) * 1152].rearrange(
                                "p (sl a two) -> p sl a two", sl=2, two=2),
                            axis=mybir.AxisListType.X)
                # transposes -> st f32[(kc), (chunk, pair, wo)]
                NP = RG // 2
                st = stp.tile([128, NCHUNK * NP * 128], F32, tag="st")
                for q in range(NP):
                    for ci in range(NCHUNK):
                        cw = min(128, KC - ci * 128)
                        ps = ps_tr.tile([128, 128], F32, tag="tr")
                        nc.tensor.transpose(
                            ps[:cw, :],
                            s2[:, q * 576 + ci * 128 : q * 576 + ci * 128 + cw],
                            identf)
                        nc.scalar.copy(
                            st[:cw, (ci * NP + q) * 128 : (ci * NP + q + 1) * 128],
                            ps[:cw, :])
                # GEMM: free dim (pair, wo, slot) = RG*128 bf16
                stb = st.bitcast(BF16)
                omm = ps_mm.tile([O, RG * 128], F32, tag="mm")
                for ci in range(NCHUNK):
                    cw = min(128, KC - ci * 128)
                    nc.tensor.matmul(
                        omm, wt_bf[:cw, ci * O : (ci + 1) * O],
                        stb[:cw, ci * NP * 256 : (ci + 1) * NP * 256],
                        start=(ci == 0), stop=(ci == NCHUNK - 1))
                # un-interleave (pair, wo, slot) -> (row, wo) + bias on Act
                osb = outp.tile([O, RG * 128], F32, tag="o")
                nc.scalar.activation(
                    osb.rearrange("o (qq sl w) -> o qq sl w", qq=NP, sl=2),
                    omm.rearrange("o (qq w sl) -> o qq sl w", qq=NP, sl=2),
                    mybir.ActivationFunctionType.Identity,
                    bias=bias_sb)
                nc.sync.dma_start(
                    out_d[:, g * RG : (g + 1) * RG, :].rearrange(
                        "o h w -> o (h w)"), osb)

    nc.compile()
    return nc


def _shard_inputs(x, offset, weight, bias):
    # weights: [(k,c), o] chunk-packed into [128, 5*64] bf16
    w = weight.reshape(O, C, K).transpose(2, 1, 0).reshape(KC, O)
    wt_p = np.zeros((128, NCHUNK * O), np.float32)
    for i in range(NCHUNK):
        cw = min(128, KC - 128 * i)
        wt_p[:cw, i * O : (i + 1) * O] = w[128 * i : 128 * i + cw]
    wt_bf = wt_p.astype(ml_dtypes.bfloat16)
    b2 = np.ascontiguousarray(bias.reshape(O, 1), np.float32)

    in_maps = []
    for core in range(8):
        b, h = divmod(core, 2)
        ylo = 64 * h - 18
        # channels-last padded x rows ylo..ylo+NY (93 rows incl +1 for r=1)
        xp = np.zeros((NY + 1, W, C), np.float32)
        src_lo, src_hi = max(0, ylo), min(H, ylo + NY + 1)
        xp[src_lo - ylo : src_hi - ylo] = x[b, :, src_lo:src_hi, :].transpose(1, 2, 0)
        tblv = np.zeros((NY, WP, C, 2), np.float32)
        tblv[:, 16 : 16 + W, :, 0] = xp[0:NY]
        tblv[:, 16 : 16 + W, :, 1] = xp[1 : NY + 1]
        tbl = np.zeros((TROWS + 2, 2 * C), ml_dtypes.bfloat16)
        tbl[:TROWS] = tblv.reshape(TROWS, 2 * C).astype(ml_dtypes.bfloat16)
        offs = offset[b, :, 64 * h : 64 * h + 64, :]  # [18, 64, 128]
        offT = np.ascontiguousarray(
            offs.transpose(2, 1, 0).reshape(W, HO2 * 18), np.float32)
        in_maps.append({"tbl": tbl, "offT": offT, "wt": wt_bf, "bias": b2})
    return in_maps


def kernel(x, offset, weight, bias):
    x = np.asarray(x, np.float32)
    offset = np.asarray(offset, np.float32)
    weight = np.asarray(weight, np.float32)
    bias = np.asarray(bias, np.float32)
    if "nc" not in _CACHE:
        _CACHE["nc"] = _build_nc()
    nc = _CACHE["nc"]
    in_maps = _shard_inputs(x, offset, weight, bias)
    res = run_bass_kernel_spmd(nc, in_maps, core_ids=list(range(8)),
                               trace=bool(_CACHE.get("trace")))
    _CACHE["exec_time_ns"] = res.exec_time_ns
    _CACHE["results"] = res
    full = np.zeros((B, O, H, W), np.float32)
    for core in range(8):
        b, h = divmod(core, 2)
        full[b, :, 64 * h : 64 * h + 64, :] = res.results[core]["out"]
    return full


if __name__ == "__main__":
    import reference as ref
    inputs = {k: np.asarray(v) for k, v in ref.setup_inputs().items()}
    out = kernel(**inputs)
    exp = np.asarray(ref.reference(**inputs))
    print("rel:", np.abs(out - exp).max() / np.abs(exp).max())


# revision 11
# speedup vs baseline: 1.1696x; 1.1696x over previous
"""Deformable Conv (DCNv1) Trainium2 Bass kernel — v2.

Problem: x[4,64,128,128], offset[4,18,128,128], weight[64,64,3,3], bias[64]
-> out[4,64,128,128].  3x3 deformable conv, stride 1, pad 1, bilinear sampling.

Sharding: 8 cores = (batch b = core//2) x (H-half h = core%2). Each core
computes out[b, :, 64h:64h+64, :].

Host prep (layout only): per-core bf16 "patch table" T[(y',x'), (c,r)] where
row (y',x') holds img rows y'-18+64h and y'-17+64h at col x'-16, channel-major
with the two rows interleaved innermost.  One 512B gather chunk starting at
row (y',x') covers the full 2x2 bilinear patch x 64 channels.  Offsets are
shipped pre-transposed as off_T[wo, (ho, ch)], weights as bf16 chunk-packed
[128, 5*64] in (k,c)-major contraction order.

Device per-core:
  1. DVE: sampling positions p = off + iota (one floor computation feeds BOTH
     the bilinear weights and the gather indices - no dual-path mismatch).
     idx32[wo, (ho,k)] = ((y0+18)*160 + x0+16)*128; w4[wo, (ho,k,x,r)] bf16.
  2. Per 4-row group: one gpsimd.indirect_dma_start gathers
     v4[wo, (row,k), 256] (4608 descriptors x 512B, HW dynamic queue).
  3. DVE blend (2x-mode layouts): m = v4*w4 (bcast over c, r innermost),
     x-fold add, r-fold add -> s2[wo, (row, k*64+c)] bf16.
  4. PE: 5 transposes/row -> st[(kc), (row,wo)]; 5 GEMMs of free dim 512
     accumulating the 576-deep contraction in PSUM; Act adds bias; DMA out.
"""

import numpy as np
import sys

sys.path.insert(0, "/opt/trn_rl_repo")

import ml_dtypes
import concourse.bacc as bacc
import concourse.mybir as mybir
from concourse import tile
from concourse.bass import AP
from concourse.bass_utils import run_bass_kernel_spmd
from concourse.masks import make_identity
from concourse.library_config import mlp

# problem constants
B, C, H, W = 4, 64, 128, 128
K, O = 9, 64
HO2 = 64            # output rows per core
NY = 92             # table y rows (image rows 64h-18 .. 64h+73)
WP = 160            # table x cols (image cols -16 .. 143)
TROWS = NY * WP
KC = K * C          # 576 im2col contraction
NCHUNK = 5          # ceil(576/128)
RG = 4              # output rows per group
NG = HO2 // RG
F32 = mybir.dt.float32
BF16 = mybir.dt.bfloat16
I32 = mybir.dt.int32
I16 = mybir.dt.int16
AX = mybir.AluOpType

_CACHE = {}


def _build_nc():
    nc = bacc.Bacc("TRN2", target_bir_lowering=False, debug=False,
                   num_swdge_queues=4)

    tbl = nc.dram_tensor("tbl", [TROWS + 2, 2 * C], BF16, kind="ExternalInput").ap()
    offT_d = nc.dram_tensor("offT", [128, HO2 * 18], F32, kind="ExternalInput").ap()
    wt_d = nc.dram_tensor("wt", [128, NCHUNK * O], BF16, kind="ExternalInput").ap()
    bias_d = nc.dram_tensor("bias", [O, 1], F32, kind="ExternalInput").ap()
    out_d = nc.dram_tensor("out", [O, HO2, W], F32, kind="ExternalOutput").ap()

    with tile.TileContext(nc) as tc:
        with (
            tc.tile_pool(name="consts", bufs=1) as consts,
            tc.tile_pool(name="wmt", bufs=10) as wmt,
            tc.tile_pool(name="v4p", bufs=5) as v4p,
            tc.tile_pool(name="mp", bufs=1) as mp,
            tc.tile_pool(name="tp", bufs=1) as tp,
            tc.tile_pool(name="sp", bufs=2) as sp,
            tc.tile_pool(name="stp", bufs=2) as stp,
            tc.tile_pool(name="outp", bufs=2) as outp,
            tc.tile_pool(name="ps_tr", bufs=4, space="PSUM") as ps_tr,
            tc.tile_pool(name="ps_mm", bufs=2, space="PSUM") as ps_mm,
        ):
            ident = consts.tile([128, 128], BF16)
            identf = consts.tile([128, 128], F32)

            offT = consts.tile([128, HO2 * 18], F32)
            nc.sync.dma_start(offT, offT_d)
            wt_bf = consts.tile([128, NCHUNK * O], BF16)
            nc.sync.dma_start(wt_bf, wt_d)
            bias_sb = consts.tile([O, 1], F32)
            nc.sync.dma_start(bias_sb, bias_d)

            # iota: cy = ki + ho + 15, cx = kj + wo + 15  (both [wo, (ho,k)])
            cy_i = wmt.tile([128, 576], I32, tag="wm")
            nc.gpsimd.iota(cy_i, pattern=[[1, 64], [1, 3], [0, 3]], base=15,
                           channel_multiplier=0)
            cy = consts.tile([128, 576], F32)
            nc.vector.tensor_copy(cy, cy_i)
            cx_i = wmt.tile([128, 576], I32, tag="wm")
            nc.gpsimd.iota(cx_i, pattern=[[0, 64], [0, 3], [1, 3]], base=15,
                           channel_multiplier=1)
            cx = consts.tile([128, 576], F32)
            nc.vector.tensor_copy(cx, cx_i)
            nc.gpsimd.load_library(mlp)
            make_identity(nc, ident)
            make_identity(nc, identf)

            # sampling positions (+16 folded into iota bases; exact floor via
            # round-to-int + is_gt correction)
            offv = offT.rearrange("p (h k two) -> p h k two", k=9, two=2)
            cyv = cy.rearrange("p (h k) -> p h k", k=9)
            cxv = cx.rearrange("p (h k) -> p h k", k=9)

            def setup(name, shape=None, dt=F32):
                return wmt.tile(shape or [128, 576], dt, tag="wm", name=name)

            pys = setup("pys")
            nc.vector.tensor_tensor(pys.rearrange("p (h k) -> p h k", k=9),
                                    offv[:, :, :, 0], cyv, AX.add)
            pxs = setup("pxs")
            nc.vector.tensor_tensor(pxs.rearrange("p (h k) -> p h k", k=9),
                                    offv[:, :, :, 1], cxv, AX.add)
            y0i = setup("y0i", dt=I32)
            nc.vector.tensor_copy(y0i, pys)
            y0f = setup("y0f")
            nc.vector.tensor_copy(y0f, y0i)
            cr = setup("cr")
            nc.vector.tensor_tensor(cr, y0f, pys, AX.is_gt)
            nc.vector.tensor_tensor(y0f, y0f, cr, AX.subtract)
            x0i = setup("x0i", dt=I32)
            nc.vector.tensor_copy(x0i, pxs)
            x0f = setup("x0f")
            nc.vector.tensor_copy(x0f, x0i)
            nc.vector.tensor_tensor(cr, x0f, pxs, AX.is_gt)
            nc.vector.tensor_tensor(x0f, x0f, cr, AX.subtract)
            ly = setup("ly")
            nc.vector.tensor_tensor(ly, pys, y0f, AX.subtract)
            lx = setup("lx")
            nc.vector.tensor_tensor(lx, pxs, x0f, AX.subtract)
            hy = setup("hy")
            nc.vector.tensor_scalar(hy, ly, -1.0, 1.0, AX.mult, AX.add)
            hx = setup("hx")
            nc.vector.tensor_scalar(hx, lx, -1.0, 1.0, AX.mult, AX.add)

            # corner weights w4[wo, (ho, k, x, r)] bf16
            w4 = consts.tile([128, 2304], BF16)
            w4v = w4.rearrange("p (a x r) -> p a x r", x=2, r=2)
            nc.vector.tensor_tensor(w4v[:, :, 0, 0], hy, hx, AX.mult)
            nc.vector.tensor_tensor(w4v[:, :, 0, 1], ly, hx, AX.mult)
            nc.vector.tensor_tensor(w4v[:, :, 1, 0], hy, lx, AX.mult)
            nc.vector.tensor_tensor(w4v[:, :, 1, 1], ly, lx, AX.mult)

            # gather indices: (y0f+2 clip [0,90])*160 + (x0f clip [0,158])
            yq = setup("yq")
            nc.vector.tensor_scalar(yq, y0f, 2.0, 0.0, AX.add, AX.max)
            nc.vector.tensor_scalar_min(yq, yq, 90.0)
            xq = setup("xq")
            nc.vector.tensor_scalar(xq, x0f, 0.0, 158.0, AX.max, AX.min)
            idxf = setup("idxf")
            nc.vector.scalar_tensor_tensor(idxf, yq, float(WP), xq,
                                           op0=AX.mult, op1=AX.add)
            # wrap for dma_gather: idx i (i = a*128 + wo, a = ho*9+k) lives at
            # wrapped[wo%16, a*8 + wo//16], replicated across 16-partition
            # groups.  Shuffle the FLOORED f32 values (single floor path),
            # then permute+convert to int16 on DVE, then replicate via DMA.
            wfd = consts.tile([16, 8 * 576], F32, name="wfd")
            for d in range(8):
                nc.sync.dma_start(wfd[:, d * 576 : (d + 1) * 576],
                                  idxf[16 * d : 16 * d + 16, :])
            idxw = consts.tile([128, 4608], I16, name="idxw")
            nc.vector.tensor_copy(
                idxw[:16, :].rearrange("p (a d) -> p a d", d=8),
                wfd.rearrange("p (d a) -> p a d", a=576))
            for rep in range(1, 8):
                nc.sync.dma_start(idxw[16 * rep : 16 * rep + 16, :],
                                  idxw[:16, :])

            # ---------------- main loop ----------------
            w4g_all = w4.rearrange("p (h k x r) -> p h k x r", k=9, x=2, r=2)
            tbl_ov = AP(tbl.tensor, 0, [[128, TROWS], [1, 256]])
            GW = RG * 9 * 8    # wrapped idx cols per group (4608/16)
            for g in range(NG):
                v4 = v4p.tile([128, RG * 9, 256], BF16, tag="v4")
                nc.gpsimd.dma_gather(
                    v4, tbl_ov, idxw[:, g * GW : (g + 1) * GW],
                    RG * 9 * 128, RG * 9 * 128, 256, elem_step=128,
                    single_packet=False, queue_num=g % 4)
                # blend: m = v4 * w4 (bcast c), chunk content = (x, c, r)
                v4v = v4.rearrange("p a (x c r) -> p a x c r", x=2, c=64, r=2)
                w4b = (w4g_all[:, g * RG : (g + 1) * RG]
                       .rearrange("p h k x r -> p (h k) x r")
                       [:, :, :, None, :].to_broadcast((128, RG * 9, 2, 64, 2)))
                m = mp.tile([128, RG * 2304], BF16, tag="m")
                mv = m.rearrange("p (a x c r) -> p a x c r", x=2, c=64, r=2)
                nc.vector.tensor_tensor(mv, v4v, w4b, AX.mult)
                # x-fold then r-fold
                t = tp.tile([128, RG * 1152], BF16, tag="t")
                tv = t.rearrange("p (a c r) -> p a c r", c=64, r=2)
                nc.vector.tensor_tensor(tv, mv[:, :, 0], mv[:, :, 1], AX.add)
                # r-fold into f32-packed row pairs: s2 f32[p,(pair,kc)],
                # bf16 view slot = row parity.  PE transposes the packed f32
                # (bit-exact, probe-verified), halving transpose count.
                s2 = sp.tile([128, (RG // 2) * 576], F32, tag="s")
                s2b = s2.bitcast(BF16)
                with nc.allow_low_precision(reason="bilinear r-fold"):
                    for q in range(RG // 2):
                        nc.vector.reduce_sum(
                            s2b[:, q * 1152 : (q + 1) * 1152].rearrange(
                                "p (kc sl) -> p sl kc", sl=2),
                            t[:, 2 * q * 1152 : (2 * q + 2) * 1152].rearrange(
                                "p (sl a two) -> p sl a two", sl=2, two=2),
                            axis=mybir.AxisListType.X)
                # transposes -> st f32[(kc), (chunk, pair, wo)]
                NP = RG // 2
                st = stp.tile([128, NCHUNK * NP * 128], F32, tag="st")
                for q in range(NP):
                    for ci in range(NCHUNK):
                        cw = min(128, KC - ci * 128)
                        ps = ps_tr.tile([128, 128], F32, tag="tr")
                        nc.tensor.transpose(
                            ps[:cw, :],
                            s2[:, q * 576 + ci * 128 : q * 576 + ci * 128 + cw],
                            identf)
                        nc.scalar.copy(
                            st[:cw, (ci * NP + q) * 128 : (ci * NP + q + 1) * 128],
                            ps[:cw, :])
                # GEMM: free dim (pair, wo, slot) = RG*128 bf16
                stb = st.bitcast(BF16)
                omm = ps_mm.tile([O, RG * 128], F32, tag="mm")
                for ci in range(NCHUNK):
                    cw = min(128, KC - ci * 128)
                    nc.tensor.matmul(
                        omm, wt_bf[:cw, ci * O : (ci + 1) * O],
                        stb[:cw, ci * NP * 256 : (ci + 1) * NP * 256],
                        start=(ci == 0), stop=(ci == NCHUNK - 1))
                # un-interleave (pair, wo, slot) -> (row, wo) + bias on Act
                osb = outp.tile([O, RG * 128], F32, tag="o")
                nc.scalar.activation(
                    osb.rearrange("o (qq sl w) -> o qq sl w", qq=NP, sl=2),
                    omm.rearrange("o (qq w sl) -> o qq sl w", qq=NP, sl=2),
                    mybir.ActivationFunctionType.Identity,
                    bias=bias_sb)
                nc.sync.dma_start(
                    out_d[:, g * RG : (g + 1) * RG, :].rearrange(
                        "o h w -> o (h w)"), osb)

    nc.compile()
    return nc


def _shard_inputs(x, offset, weight, bias):
    # weights: [(k,c), o] chunk-packed into [128, 5*64] bf16
    w = weight.reshape(O, C, K).transpose(2, 1, 0).reshape(KC, O)
    wt_p = np.zeros((128, NCHUNK * O), np.float32)
    for i in range(NCHUNK):
        cw = min(128, KC - 128 * i)
        wt_p[:cw, i * O : (i + 1) * O] = w[128 * i : 128 * i + cw]
    wt_bf = wt_p.astype(ml_dtypes.bfloat16)
    b2 = np.ascontiguousarray(bias.reshape(O, 1), np.float32)

    in_maps = []
    for core in range(8):
        b, h = divmod(core, 2)
        ylo = 64 * h - 18
        # channels-last padded x rows ylo..ylo+NY (93 rows incl +1 for r=1)
        xp = np.zeros((NY + 1, W, C), np.float32)
        src_lo, src_hi = max(0, ylo), min(H, ylo + NY + 1)
        xp[src_lo - ylo : src_hi - ylo] = x[b, :, src_lo:src_hi, :].transpose(1, 2, 0)
        tblv = np.zeros((NY, WP, C, 2), np.float32)
        tblv[:, 16 : 16 + W, :, 0] = xp[0:NY]
        tblv[:, 16 : 16 + W, :, 1] = xp[1 : NY + 1]
        tbl = np.zeros((TROWS + 2, 2 * C), ml_dtypes.bfloat16)
        tbl[:TROWS] = tblv.reshape(TROWS, 2 * C).astype(ml_dtypes.bfloat16)
        offs = offset[b, :, 64 * h : 64 * h + 64, :]  # [18, 64, 128]
        offT = np.ascontiguousarray(
            offs.transpose(2, 1, 0).reshape(W, HO2 * 18), np.float32)
        in_maps.append({"tbl": tbl, "offT": offT, "wt": wt_bf, "bias": b2})
    return in_maps


def kernel(x, offset, weight, bias):
    x = np.asarray(x, np.float32)
    offset = np.asarray(offset, np.float32)
    weight = np.asarray(weight, np.float32)
    bias = np.asarray(bias, np.float32)
    if "nc" not in _CACHE:
        _CACHE["nc"] = _build_nc()
    nc = _CACHE["nc"]
    in_maps = _shard_inputs(x, offset, weight, bias)
    res = run_bass_kernel_spmd(nc, in_maps, core_ids=list(range(8)),
                               trace=bool(_CACHE.get("trace")))
    _CACHE["exec_time_ns"] = res.exec_time_ns
    _CACHE["results"] = res
    full = np.zeros((B, O, H, W), np.float32)
    for core in range(8):
        b, h = divmod(core, 2)
        full[b, :, 64 * h : 64 * h + 64, :] = res.results[core]["out"]
    return full


if __name__ == "__main__":
    import reference as ref
    inputs = {k: np.asarray(v) for k, v in ref.setup_inputs().items()}
    out = kernel(**inputs)
    exp = np.asarray(ref.reference(**inputs))
    print("rel:", np.abs(out - exp).max() / np.abs(exp).max())
